# revision 13
# baseline (speedup 1.0000x reference)
"""AdaptiveGridKANLayer on 8 TRN2 NeuronCores.

out[b,o] = sum_i sum_g exp(-((x[b,i]-c_g)/w)^2) * coeffs[o,i,g]
         + sum_i silu(x[b,i]) * base_w[o,i]

B=65536, in=out=128, G=8, centers = linspace(-1,1,8), w = 2/7.

Strategy (data-parallel over batch, weights replicated):
- Host: transpose x to feature-major [128, B], shard columns 8 ways; fold the
  Gaussian factorization constants e^(7g-g^2) into the coeffs; output comes
  back transposed and is untransposed on host.
- Device, per core (u = (x+1)/w): basis_g = e^(-(u-g)^2) = p * s^g * const
  with p = exp(-u^2) (ScalarE Square+Exp), s = exp(7x) (ScalarE Exp).
  VectorE builds the power chain t_g = t_{g-1} * s (7 bf16 2x-mode
  multiplies per chunk); TensorE contracts 9 K-tiles (8 Gaussian + silu)
  as [128,128]x[128,512] bf16 matmuls accumulating into f32 PSUM.
- silu lives in a different activation-table set than exp, so all Exp/Square
  passes run first, then one table switch, then all Silu passes (gated via a
  bias tile so the scheduler cannot interleave the phases); the silu matmuls
  close each PSUM accumulation group afterwards.
- All TensorE matmuls are chained in a strict program order and redundant
  LDWEIGHTS are dropped (ldweights=False) when consecutive matmuls share the
  stationary operand.
"""

import numpy as np

BATCH = 65536
GRID = 8
NCORES = 8
BLOC = BATCH // NCORES  # 8192 batch columns per core
FDE = 2048  # elementwise chunk
NCH = BLOC // FDE
FDP = 1024  # psum sub-chunk
NSUB = BLOC // FDP
MMF = 512  # matmul free dim (one PSUM bank)
W = 2.0 / (GRID - 1)

_NC = None


def _build():
    import concourse.mybir as mybir
    from concourse import bacc
    from concourse.tile import TileContext, add_dep_helper

    AF = mybir.ActivationFunctionType
    bf16 = mybir.dt.bfloat16
    f32 = mybir.dt.float32

    nc = bacc.Bacc("TRN2", num_devices=NCORES)
    cst = nc.alloc_sbuf_tensor("const-float32-bias-c", [128, 1], f32)
    nc.gpsimd.memset(cst.ap(), 1.0 / W)
    nc.const_aps.aps[(f32, 1.0 / W)] = cst.ap()
    nc.all_engine_barrier()
    xt = nc.dram_tensor("xt", [128, BLOC], f32, kind="ExternalInput").ap()
    wt = nc.dram_tensor("wt", [128, 9 * 128], bf16, kind="ExternalInput").ap()
    out = nc.dram_tensor("out", [128, BLOC], f32, kind="ExternalOutput").ap()

    with TileContext(nc) as tc:
        with (
            tc.tile_pool(name="const", bufs=1) as cpool,
            tc.tile_pool(name="work", bufs=2) as wpool,
            tc.tile_pool(name="obuf", bufs=3) as opool,
            tc.tile_pool(name="psum", bufs=4, space="PSUM") as ppool,
        ):
            x_all = cpool.tile([128, BLOC], f32)
            for c in range(NCH):
                cs = slice(c * FDE, (c + 1) * FDE)
                nc.sync.dma_start(x_all[:, cs], xt[:, cs])
            w_sb = cpool.tile([128, 9, 128], bf16)
            nc.sync.dma_start(w_sb[:], wt.rearrange("p (g o) -> p g o", g=9))

            mm_chain = []  # (raw_inst, weight_key) in required PE order

            def mm(psum_ap, wg, rhs_ap, start, stop):
                h = nc.tensor.matmul(
                    psum_ap, w_sb[:, wg, :], rhs_ap, start=start, stop=stop
                )
                mm_chain.append((h.ins, wg))

            # Phase 1 (exp_and_others): s, q, t0 + vector chain + gauss MMs.
            psums = []
            last_t0 = None
            gauss_end_idx = []  # chain length after each sub-chunk's gauss MMs
            for c in range(NCH):
                cs = slice(c * FDE, (c + 1) * FDE)
                xc = x_all[:, cs]
                s = wpool.tile([128, FDE], bf16, tag="s")
                nc.scalar.activation(s[:], xc, AF.Exp, scale=2.0 / W)
                q = wpool.tile([128, FDE], f32, tag="q")
                nc.scalar.activation(q[:], xc, AF.Square, bias=1.0 / W, scale=1.0 / W)
                t0 = wpool.tile([128, FDE], bf16, tag="t0")
                last_t0 = nc.scalar.activation(t0[:], q[:], AF.Exp, scale=-1.0)
                tg = [t0]
                for g in range(1, GRID):
                    t = wpool.tile([128, FDE], bf16, tag=f"t{g}")
                    nc.vector.tensor_mul(t[:], tg[-1][:], s[:])
                    tg.append(t)
                for half in range(FDE // FDP):
                    psum = ppool.tile([128, FDP], f32)
                    psums.append(psum)
                    for g in range(GRID):
                        for n in range(FDP // MMF):
                            lo = half * FDP + n * MMF
                            mm(
                                psum[:, n * MMF : (n + 1) * MMF],
                                g,
                                tg[g][:, lo : lo + MMF],
                                start=(g == 0),
                                stop=False,
                            )
                    gauss_end_idx.append(len(mm_chain))

            # Phase 2 (silu_and_others), gated on the last exp-phase act via a
            # bias tile so the two table phases cannot interleave on ScalarE.
            gate = cpool.tile([128, 1], f32)
            gate_op = nc.scalar.activation(
                gate[:], x_all[:, BLOC - 1 : BLOC], AF.Identity, scale=0.0
            )
            add_dep_helper(gate_op.ins, last_t0.ins, False, "table phase order")
            silu_all = cpool.tile([128, BLOC], bf16)
            for c in range(NCH):
                cs = slice(c * FDE, (c + 1) * FDE)
                nc.scalar.activation(silu_all[:, cs], x_all[:, cs], AF.Silu, bias=gate[:])

            # Silu MMs close each psum group; copies + out DMA follow.
            silu_mms = []
            for k in range(NSUB):
                grp = []
                for n in range(FDP // MMF):
                    lo = k * FDP + n * MMF
                    h = nc.tensor.matmul(
                        psums[k][:, n * MMF : (n + 1) * MMF],
                        w_sb[:, 8, :],
                        silu_all[:, lo : lo + MMF],
                        start=False,
                        stop=True,
                    )
                    grp.append((h.ins, 8))
                silu_mms.append(grp)
                ob = opool.tile([128, FDP], f32, tag="ob")
                if k < 4:
                    nc.scalar.copy(ob[:], psums[k][:])
                else:
                    nc.vector.tensor_copy(ob[:], psums[k][:])
                nc.sync.dma_start(out[:, k * FDP : (k + 1) * FDP], ob[:])

            # Assemble the strict PE order:
            # G0 G1 G2 G3 S0 G4 S1 G5 S2 G6 S3 G7 S4 S5 S6 S7
            chain = list(mm_chain[: gauss_end_idx[3]])
            for k in range(4, NSUB):
                chain += silu_mms[k - 4]
                chain += mm_chain[gauss_end_idx[k - 1] : gauss_end_idx[k]]
            for k in range(NSUB - 4, NSUB):
                chain += silu_mms[k]
            import os

            mode = os.environ.get("KAN_PE_MODE", "chain+dedupe")
            if mode != "none":
                prev_inst, prev_key = chain[0]
                for inst, key in chain[1:]:
                    add_dep_helper(inst, prev_inst, False, "pe order")
                    if "dedupe" in mode and key == prev_key:
                        inst.ldweights = False
                    prev_inst, prev_key = inst, key

    nc.compile()
    return nc


def _prep_weights(coeffs, base_w):
    import ml_dtypes

    g = np.arange(GRID, dtype=np.float64)
    K = np.exp(7.0 * g - g * g)  # t_g = basis_g * e^(g^2-7g) -> fold inverse
    blocks = [
        (coeffs[:, :, gi].astype(np.float64) * K[gi]).T for gi in range(GRID)
    ]  # [in, out] each
    blocks.append(base_w.astype(np.float64).T)
    wtm = np.concatenate(blocks, axis=1)  # [128, 9*128]
    return np.ascontiguousarray(wtm.astype(ml_dtypes.bfloat16))


def kernel(x, coeffs, base_w, centers):
    from concourse.bass_utils import run_bass_kernel_spmd

    global _NC
    if _NC is None:
        _NC = _build()

    wtm = _prep_weights(coeffs, base_w)
    xT = np.ascontiguousarray(np.asarray(x, dtype=np.float32).T)  # [128, B]
    in_maps = [
        {
            "xt": np.ascontiguousarray(xT[:, c * BLOC : (c + 1) * BLOC]),
            "wt": wtm,
        }
        for c in range(NCORES)
    ]
    res = run_bass_kernel_spmd(_NC, in_maps, list(range(NCORES)))
    outT = np.concatenate([res.results[c]["out"] for c in range(NCORES)], axis=1)
    return np.ascontiguousarray(outT.T)


# revision 15
# speedup vs baseline: 1.2215x; 1.2215x over previous
"""AdaptiveGridKANLayer on 8 TRN2 NeuronCores.

out[b,o] = sum_i sum_g exp(-((x[b,i]-c_g)/w)^2) * coeffs[o,i,g]
         + sum_i silu(x[b,i]) * base_w[o,i]

B=65536, in=out=128, G=8, centers = linspace(-1,1,8), w = 2/7.

Strategy (data-parallel over batch, weights replicated):
- Host: transpose x to feature-major [128, B], shard columns 8 ways; fold the
  Gaussian factorization constants e^(7g-g^2) into the coeffs; output comes
  back transposed and is untransposed on host.
- Device, per core (u = (x+1)/w): basis_g = e^(-(u-g)^2) = p * s^g * const
  with p = exp(-u^2) (ScalarE Square+Exp), s = exp(7x) (ScalarE Exp).
  VectorE builds the power chain t_g = t_{g-1} * s (bf16 2x-mode
  multiplies); TensorE contracts 9 K-tiles (8 Gaussian + silu) as
  [128,128]x[128,512] bf16 matmuls accumulating into f32 PSUM.
- silu lives in a different activation-table set than exp, so all Exp/Square
  passes run first, then one table switch, then all Silu passes (gated via a
  bias tile so the scheduler cannot interleave the phases); the silu matmuls
  close each PSUM accumulation group, then PSUM is copied out (copies split
  between ScalarE and VectorE) and DMAed.
- The first chunk's elementwise work runs at half width so the vector chain
  starts as early as possible.
"""

import numpy as np

BATCH = 65536
GRID = 8
NCORES = 8
BLOC = BATCH // NCORES  # 8192 batch columns per core
FDE = 2048  # elementwise chunk
NCH = BLOC // FDE
FDP = 1024  # psum sub-chunk
NSUB = BLOC // FDP
MMF = 512  # matmul free dim (one PSUM bank)
W = 2.0 / (GRID - 1)

_NC = None


def _build():
    import concourse.mybir as mybir
    from concourse import bacc
    from concourse.tile import TileContext, add_dep_helper

    AF = mybir.ActivationFunctionType
    bf16 = mybir.dt.bfloat16
    f32 = mybir.dt.float32

    nc = bacc.Bacc("TRN2", num_devices=NCORES)
    cst = nc.alloc_sbuf_tensor("const-float32-bias-c", [128, 1], f32)
    nc.gpsimd.memset(cst.ap(), 1.0 / W)
    nc.const_aps.aps[(f32, 1.0 / W)] = cst.ap()
    nc.all_engine_barrier()
    xt = nc.dram_tensor("xt", [128, BLOC], f32, kind="ExternalInput").ap()
    wt = nc.dram_tensor("wt", [128, 9 * 128], bf16, kind="ExternalInput").ap()
    out = nc.dram_tensor("out", [128, BLOC], f32, kind="ExternalOutput").ap()

    with TileContext(nc) as tc:
        with (
            tc.tile_pool(name="const", bufs=1) as cpool,
            tc.tile_pool(name="work", bufs=2) as wpool,
            tc.tile_pool(name="obuf", bufs=3) as opool,
            tc.tile_pool(name="psum", bufs=4, space="PSUM") as ppool,
        ):
            x_all = cpool.tile([128, BLOC], f32)
            for k in range(NSUB):
                ks = slice(k * FDP, (k + 1) * FDP)
                nc.sync.dma_start(x_all[:, ks], xt[:, ks])
            w_sb = cpool.tile([128, 9, 128], bf16)
            nc.sync.dma_start(w_sb[:], wt.rearrange("p (g o) -> p g o", g=9))

            # Phase 1 (exp_and_others): s, q, t0 + vector chain + gauss MMs.
            psums = []
            last_t0 = None
            for c in range(NCH):
                s = wpool.tile([128, FDE], bf16, tag="s")
                q = wpool.tile([128, FDE], f32, tag="q")
                t0 = wpool.tile([128, FDE], bf16, tag="t0")
                tg = [t0] + [
                    wpool.tile([128, FDE], bf16, tag=f"t{g}", name=f"t{g}_{c}")
                    for g in range(1, GRID)
                ]
                # chunk 0 runs at half width so the chain starts earlier
                nparts = 2 if c == 0 else 1
                fd = FDE // nparts
                for h in range(nparts):
                    hs = slice(h * fd, (h + 1) * fd)
                    xc = x_all[:, c * FDE + h * fd : c * FDE + (h + 1) * fd]
                    nc.scalar.activation(s[:, hs], xc, AF.Exp, scale=2.0 / W)
                    nc.scalar.activation(
                        q[:, hs], xc, AF.Square, bias=1.0 / W, scale=1.0 / W
                    )
                    last_t0 = nc.scalar.activation(
                        t0[:, hs], q[:, hs], AF.Exp, scale=-1.0
                    )
                    for g in range(1, GRID):
                        nc.vector.tensor_mul(tg[g][:, hs], tg[g - 1][:, hs], s[:, hs])
                for half in range(FDE // FDP):
                    psum = ppool.tile([128, FDP], f32)
                    psums.append(psum)
                    for g in range(GRID):
                        for n in range(FDP // MMF):
                            lo = half * FDP + n * MMF
                            nc.tensor.matmul(
                                psum[:, n * MMF : (n + 1) * MMF],
                                w_sb[:, g, :],
                                tg[g][:, lo : lo + MMF],
                                start=(g == 0),
                                stop=False,
                            )

            # Phase 2 (silu_and_others), gated on the last exp-phase act via a
            # bias tile so the two table phases cannot interleave on ScalarE.
            gate = cpool.tile([128, 1], f32)
            gate_op = nc.scalar.activation(
                gate[:], x_all[:, BLOC - 1 : BLOC], AF.Identity, scale=0.0
            )
            add_dep_helper(gate_op.ins, last_t0.ins, False, "table phase order")
            silu_all = cpool.tile([128, BLOC], bf16)
            scalar_copy = {0, 1, 2, 3, 4, 6}
            for c in range(NCH):
                cs = slice(c * FDE, (c + 1) * FDE)
                nc.scalar.activation(
                    silu_all[:, cs], x_all[:, cs], AF.Silu, bias=gate[:]
                )
                for half in range(FDE // FDP):
                    k = c * 2 + half
                    for n in range(FDP // MMF):
                        lo = k * FDP + n * MMF
                        nc.tensor.matmul(
                            psums[k][:, n * MMF : (n + 1) * MMF],
                            w_sb[:, 8, :],
                            silu_all[:, lo : lo + MMF],
                            start=False,
                            stop=True,
                        )
                    ob = opool.tile([128, FDP], f32, tag="ob")
                    if k in scalar_copy:
                        nc.scalar.copy(ob[:], psums[k][:])
                    else:
                        nc.vector.tensor_copy(ob[:], psums[k][:])
                    nc.sync.dma_start(out[:, k * FDP : (k + 1) * FDP], ob[:])

    nc.compile()
    return nc


def _prep_weights(coeffs, base_w):
    import ml_dtypes

    g = np.arange(GRID, dtype=np.float64)
    K = np.exp(7.0 * g - g * g)  # t_g = basis_g * e^(g^2-7g) -> fold inverse
    blocks = [
        (coeffs[:, :, gi].astype(np.float64) * K[gi]).T for gi in range(GRID)
    ]  # [in, out] each
    blocks.append(base_w.astype(np.float64).T)
    wtm = np.concatenate(blocks, axis=1)  # [128, 9*128]
    return np.ascontiguousarray(wtm.astype(ml_dtypes.bfloat16))


def kernel(x, coeffs, base_w, centers):
    from concourse.bass_utils import run_bass_kernel_spmd

    global _NC
    if _NC is None:
        _NC = _build()

    wtm = _prep_weights(coeffs, base_w)
    xT = np.ascontiguousarray(np.asarray(x, dtype=np.float32).T)  # [128, B]
    in_maps = [
        {
            "xt": np.ascontiguousarray(xT[:, c * BLOC : (c + 1) * BLOC]),
            "wt": wtm,
        }
        for c in range(NCORES)
    ]
    res = run_bass_kernel_spmd(_NC, in_maps, list(range(NCORES)))
    outT = np.concatenate([res.results[c]["out"] for c in range(NCORES)], axis=1)
    return np.ascontiguousarray(outT.T)


# revision 18
# speedup vs baseline: 1.2298x; 1.0069x over previous
"""AdaptiveGridKANLayer on 8 TRN2 NeuronCores.

out[b,o] = sum_i sum_g exp(-((x[b,i]-c_g)/w)^2) * coeffs[o,i,g]
         + sum_i silu(x[b,i]) * base_w[o,i]

B=65536, in=out=128, G=8, centers = linspace(-1,1,8), w = 2/7.

Strategy (data-parallel over batch, weights replicated):
- Host: transpose x to feature-major [128, B], shard columns 8 ways; fold the
  Gaussian factorization constants e^(7g-g^2) into the coeffs; output comes
  back transposed and is untransposed on host.
- Device, per core (u = (x+1)/w): basis_g = e^(-(u-g)^2) = p * s^g * const
  with p = exp(-u^2) (ScalarE Square+Exp), s = exp(7x) (ScalarE Exp).
  VectorE builds the power chain t_g = t_{g-1} * s (bf16 2x-mode
  multiplies); TensorE contracts 9 K-tiles (8 Gaussian + silu) as
  [128,128]x[128,512] bf16 matmuls accumulating into f32 PSUM.
- silu lives in a different activation-table set than exp, so all Exp/Square
  passes run first, then one table switch, then all Silu passes (gated via a
  bias tile so the scheduler cannot interleave the phases); the silu matmuls
  close each PSUM accumulation group, then PSUM is copied out (copies split
  between ScalarE and VectorE) and DMAed.
- The first chunk's elementwise work runs at half width so the vector chain
  starts as early as possible.
"""

import numpy as np

BATCH = 65536
GRID = 8
NCORES = 8
BLOC = BATCH // NCORES  # 8192 batch columns per core
FDE = 2048  # elementwise chunk
NCH = BLOC // FDE
FDP = 1024  # psum sub-chunk
NSUB = BLOC // FDP
MMF = 512  # matmul free dim (one PSUM bank)
W = 2.0 / (GRID - 1)

_NC = None


def _build():
    import concourse.mybir as mybir
    from concourse import bacc
    from concourse.tile import TileContext, add_dep_helper

    AF = mybir.ActivationFunctionType
    bf16 = mybir.dt.bfloat16
    f32 = mybir.dt.float32

    nc = bacc.Bacc("TRN2", num_devices=NCORES)
    cst = nc.alloc_sbuf_tensor("const-float32-bias-c", [128, 1], f32)
    nc.gpsimd.memset(cst.ap(), 1.0 / W)
    nc.const_aps.aps[(f32, 1.0 / W)] = cst.ap()
    nc.all_engine_barrier()
    xt = nc.dram_tensor("xt", [128, BLOC], f32, kind="ExternalInput").ap()
    wt = nc.dram_tensor("wt", [128, 9 * 128], bf16, kind="ExternalInput").ap()
    out = nc.dram_tensor("out", [128, BLOC], f32, kind="ExternalOutput").ap()

    with TileContext(nc) as tc:
        with (
            tc.tile_pool(name="const", bufs=1) as cpool,
            tc.tile_pool(name="work", bufs=2) as wpool,
            tc.tile_pool(name="obuf", bufs=3) as opool,
            tc.tile_pool(name="psum", bufs=4, space="PSUM") as ppool,
        ):
            # weights go through SWDGE so they don't queue behind the x
            # pieces on the sync HWDGE FIFO
            w_sb = cpool.tile([128, 9, 128], bf16)
            nc.gpsimd.dma_start(w_sb[:], wt.rearrange("p (g o) -> p g o", g=9))
            x_all = cpool.tile([128, BLOC], f32)
            for k in range(NSUB):
                ks = slice(k * FDP, (k + 1) * FDP)
                nc.sync.dma_start(x_all[:, ks], xt[:, ks])

            # Phase 1 (exp_and_others): s, q, t0 + vector chain + gauss MMs.
            psums = []
            last_t0 = None
            for c in range(NCH):
                s = wpool.tile([128, FDE], bf16, tag="s", bufs=3)
                q = wpool.tile([128, FDE], f32, tag="q")
                t0 = wpool.tile([128, FDE], bf16, tag="t0", bufs=3)
                tg = [t0] + [
                    wpool.tile([128, FDE], bf16, tag=f"t{g}", name=f"t{g}_{c}")
                    for g in range(1, GRID)
                ]
                # chunk 0 runs at half width so the chain starts earlier
                nparts = 2 if c == 0 else 1
                fd = FDE // nparts
                for h in range(nparts):
                    hs = slice(h * fd, (h + 1) * fd)
                    xc = x_all[:, c * FDE + h * fd : c * FDE + (h + 1) * fd]
                    nc.scalar.activation(s[:, hs], xc, AF.Exp, scale=2.0 / W)
                    nc.scalar.activation(
                        q[:, hs], xc, AF.Square, bias=1.0 / W, scale=1.0 / W
                    )
                    last_t0 = nc.scalar.activation(
                        t0[:, hs], q[:, hs], AF.Exp, scale=-1.0
                    )
                    for g in range(1, GRID):
                        nc.vector.tensor_mul(tg[g][:, hs], tg[g - 1][:, hs], s[:, hs])
                for half in range(FDE // FDP):
                    psum = ppool.tile([128, FDP], f32)
                    psums.append(psum)
                    for g in range(GRID):
                        for n in range(FDP // MMF):
                            lo = half * FDP + n * MMF
                            nc.tensor.matmul(
                                psum[:, n * MMF : (n + 1) * MMF],
                                w_sb[:, g, :],
                                tg[g][:, lo : lo + MMF],
                                start=(g == 0),
                                stop=False,
                            )

            # Phase 2 (silu_and_others), gated on the last exp-phase act via a
            # bias tile so the two table phases cannot interleave on ScalarE.
            gate = cpool.tile([128, 1], f32)
            gate_op = nc.scalar.activation(
                gate[:], x_all[:, BLOC - 1 : BLOC], AF.Identity, scale=0.0
            )
            add_dep_helper(gate_op.ins, last_t0.ins, True, "table phase order")
            silu_all = cpool.tile([128, BLOC], bf16)
            scalar_copy = {0, 1, 2, 3, 4, 6}
            for c in range(NCH):
                cs = slice(c * FDE, (c + 1) * FDE)
                nc.scalar.activation(
                    silu_all[:, cs], x_all[:, cs], AF.Silu, bias=gate[:]
                )
                for half in range(FDE // FDP):
                    k = c * 2 + half
                    for n in range(FDP // MMF):
                        lo = k * FDP + n * MMF
                        nc.tensor.matmul(
                            psums[k][:, n * MMF : (n + 1) * MMF],
                            w_sb[:, 8, :],
                            silu_all[:, lo : lo + MMF],
                            start=False,
                            stop=True,
                        )
                    ob = opool.tile([128, FDP], f32, tag="ob")
                    if k in scalar_copy:
                        nc.scalar.copy(ob[:], psums[k][:])
                    else:
                        nc.vector.tensor_copy(ob[:], psums[k][:])
                    nc.sync.dma_start(out[:, k * FDP : (k + 1) * FDP], ob[:])

    nc.compile()
    return nc


def _prep_weights(coeffs, base_w):
    import ml_dtypes

    g = np.arange(GRID, dtype=np.float64)
    K = np.exp(7.0 * g - g * g)  # t_g = basis_g * e^(g^2-7g) -> fold inverse
    blocks = [
        (coeffs[:, :, gi].astype(np.float64) * K[gi]).T for gi in range(GRID)
    ]  # [in, out] each
    blocks.append(base_w.astype(np.float64).T)
    wtm = np.concatenate(blocks, axis=1)  # [128, 9*128]
    return np.ascontiguousarray(wtm.astype(ml_dtypes.bfloat16))


def kernel(x, coeffs, base_w, centers):
    from concourse.bass_utils import run_bass_kernel_spmd

    global _NC
    if _NC is None:
        _NC = _build()

    wtm = _prep_weights(coeffs, base_w)
    xT = np.ascontiguousarray(np.asarray(x, dtype=np.float32).T)  # [128, B]
    in_maps = [
        {
            "xt": np.ascontiguousarray(xT[:, c * BLOC : (c + 1) * BLOC]),
            "wt": wtm,
        }
        for c in range(NCORES)
    ]
    res = run_bass_kernel_spmd(_NC, in_maps, list(range(NCORES)))
    outT = np.concatenate([res.results[c]["out"] for c in range(NCORES)], axis=1)
    return np.ascontiguousarray(outT.T)
